# revision 12
# baseline (speedup 1.0000x reference)
"""DiffMoE MLP (expert-choice routing) Trainium2 kernel.

Model (reference semantics):
  x (4,2048,1024) -> flatten (8192,1024) tokens
  scores = (tanh(x @ gate_w.T)+1)/2            (8192, 8)
  per-expert top-k token selection, k=1024 (expert-choice)
  cap_loss = BCE(cap_mlp(x), keep_mask).mean()
  out = x; out[sel_e] += score * (gelu(LN(x[sel_e]) @ fc1[e].T + b1) @ fc2[e].T + b2)

Distribution (8 NeuronCores):
  - expert-parallel MLP: core e owns expert e's fc1/fc2 (host-sliced inputs)
  - token-parallel scores + capacity predictor: core r owns token slice r
  - AllToAll exchanges scores so core e holds all 8192 scores of expert e
  - top-k via exact fp32 threshold bisection (count >= k), compaction via
    gpsimd sparse_gather, dispatch via dma_gather, combine via local
    dma_scatter_add into a dense buffer + ReduceScatter(add) + residual.

Everything of the reference computation runs on-device. Host only shards
(slices/transposes of weights), concatenates output slices, and supplies
trivial constants (identity/ones/iota).
"""

import numpy as np

import concourse.bass as bass
import concourse.tile as tile
import concourse.mybir as mybir
from concourse import bacc
from concourse import library_config
from concourse.bass_utils import run_bass_kernel_spmd

F32 = mybir.dt.float32
BF16 = mybir.dt.bfloat16
I16 = mybir.dt.int16
I8 = mybir.dt.int8
U32 = mybir.dt.uint32
AF = mybir.ActivationFunctionType
ALU = mybir.AluOpType

# shapes
D = 1024
DD = 4096
E = 8
BS = 8192
SL = 1024          # tokens per core (slice)
K = 1024           # capacity per expert
NT = SL // 128     # 8 token tiles per slice
ND = D // 128      # 8 d tiles
NF = DD // 128     # 32 f tiles
LN_EPS = 1e-5
BISECT_ITERS = 24

_CACHE = {}


def _dep(later, earlier):
    """Order `later` after `earlier` on the same engine stream."""
    tile.add_dep_helper(later.ins, earlier.ins, sync=True)


def build_program():
    nc = bacc.Bacc("TRN2", target_bir_lowering=False, debug=False, num_devices=8)

    # ---------------- I/O ----------------
    xs = nc.dram_tensor("xs", [SL, D], F32, kind="ExternalInput")
    xfull = nc.dram_tensor("xfull", [BS, D], F32, kind="ExternalInput")
    gwt = nc.dram_tensor("gwt", [D, E], F32, kind="ExternalInput")
    w1t = nc.dram_tensor("w1t", [D, DD], F32, kind="ExternalInput")
    w2t = nc.dram_tensor("w2t", [DD, D], F32, kind="ExternalInput")
    b1d = nc.dram_tensor("b1d", [DD], F32, kind="ExternalInput")
    b2d = nc.dram_tensor("b2d", [D], F32, kind="ExternalInput")
    cw1t = nc.dram_tensor("cw1t", [D, D], F32, kind="ExternalInput")
    cb1d = nc.dram_tensor("cb1d", [D], F32, kind="ExternalInput")
    cw2t = nc.dram_tensor("cw2t", [D, E], F32, kind="ExternalInput")
    cb2d = nc.dram_tensor("cb2d", [E], F32, kind="ExternalInput")
    nwd = nc.dram_tensor("nwd", [D], F32, kind="ExternalInput")
    nbd = nc.dram_tensor("nbd", [D], F32, kind="ExternalInput")
    identd = nc.dram_tensor("identd", [128, 128], F32, kind="ExternalInput")
    onesd = nc.dram_tensor("onesd", [128, 128], F32, kind="ExternalInput")
    iotad = nc.dram_tensor("iotad", [16, 512], I16, kind="ExternalInput")

    out_slice = nc.dram_tensor("out_slice", [SL, D], F32, kind="ExternalOutput")
    loss = nc.dram_tensor("loss", [1, 1], F32, kind="ExternalOutput")

    # internal dram
    a2a_in = nc.dram_tensor("a2a_in", [E, SL], F32)
    a2a_out = nc.dram_tensor("a2a_out", [E, SL], F32)
    tau_in = nc.dram_tensor("tau_in", [1, 1], F32)
    tau_out = nc.dram_tensor("tau_out", [E, 1], F32)
    idx_d = nc.dram_tensor("idx_d", [K], I16)
    sel_d = nc.dram_tensor("sel_d", [K], F32)
    delta = nc.dram_tensor("delta", [BS, D], F32)
    delta_rs = nc.dram_tensor("delta_rs", [SL, D], F32)
    loss_in = nc.dram_tensor("loss_in", [1, 1], F32)
    loss_out = nc.dram_tensor("loss_out", [1, 1], F32)

    groups = [list(range(8))]

    with tile.TileContext(nc) as tc:
        with tc.tile_pool(name="smalls", bufs=1) as smalls:
            gw_sb = smalls.tile([128, ND, E], F32)
            b1col = smalls.tile([128, NF], F32)
            gcol = smalls.tile([128, ND], F32)
            bcol = smalls.tile([128, ND], F32)
            cb1col = smalls.tile([128, ND], F32)
            cb2sb = smalls.tile([E, 1], F32)
            b2rowb = smalls.tile([1, D], BF16)
            ident_sb = smalls.tile([128, 128], F32)
            ones_sb = smalls.tile([128, 128], F32)
            iota_sb = smalls.tile([16, 512], I16)
            scT_sl = smalls.tile([E, SL], F32)
            logitsT = smalls.tile([E, SL], F32)
            sc_own_b = smalls.tile([128, 64], F32)
            sc16 = smalls.tile([16, 512], F32)
            tau_sb = smalls.tile([E, 1], F32)
            idx128 = smalls.tile([128, 64], I16)
            s_rowb = smalls.tile([1, SL], BF16)
            s_bc = smalls.tile([128, SL], BF16)

            # const loads
            nc.sync.dma_start(ident_sb[:], identd[:, :])
            nc.sync.dma_start(ones_sb[:], onesd[:, :])
            nc.sync.dma_start(iota_sb[:], iotad[:, :])
            nc.sync.dma_start(gw_sb[:], gwt.ap().rearrange("(dt p) e -> p dt e", p=128))
            nc.sync.dma_start(b1col[:], b1d.ap().rearrange("(f p) -> p f", p=128))
            nc.sync.dma_start(gcol[:], nwd.ap().rearrange("(f p) -> p f", p=128))
            nc.sync.dma_start(bcol[:], nbd.ap().rearrange("(f p) -> p f", p=128))
            nc.sync.dma_start(cb1col[:], cb1d.ap().rearrange("(f p) -> p f", p=128))
            nc.sync.dma_start(cb2sb[:], cb2d.ap().unsqueeze(1))

            # ---------------- phase 0a: scores + cap mlp + bisect ------------
            bis_cm = tc.tile_pool(name="bis", bufs=1, side="right")
            bis = bis_cm.__enter__()
            xln_cm = tc.tile_pool(name="xlnpool", bufs=1)
            xlnpool = xln_cm.__enter__()
            xlnT = xlnpool.tile([128, ND, K], BF16)

            with (
                tc.tile_pool(name="stage", bufs=2) as stage,
                tc.tile_pool(name="p0", bufs=2) as p0,
                tc.tile_pool(name="cappool", bufs=1) as cappool,
                tc.tile_pool(name="psA", bufs=1, space="PSUM") as psA,
                tc.tile_pool(name="psT", bufs=2, space="PSUM") as psT,
                tc.tile_pool(name="psC", bufs=2, space="PSUM") as psC,
            ):
                # zero the dense combine buffer early (overlaps everything)
                zt = stage.tile([128, 4096], F32, tag="st", bufs=2)
                nc.vector.memset(zt[:], 0.0)
                for c in range(16):
                    nc.sync.dma_start(
                        delta.ap()[c * 512:(c + 1) * 512, :], zt[:]
                    )

                CW1T = cappool.tile([128, ND, D], BF16)     # 16 KB/part
                for dt in range(ND):
                    st = stage.tile([128, D], F32, tag="st", bufs=2)
                    nc.sync.dma_start(st[:], cw1t.ap()[dt * 128:(dt + 1) * 128, :])
                    nc.vector.tensor_copy(CW1T[:, dt, :], st[:])
                cw2b = cappool.tile([128, ND, E], BF16)
                stc = stage.tile([128, ND, E], F32, tag="st", bufs=2)
                nc.sync.dma_start(stc[:], cw2t.ap().rearrange("(it p) e -> p it e", p=128))
                nc.vector.tensor_copy(cw2b[:], stc[:])
                stb = stage.tile([1, D], F32, tag="st", bufs=2)
                nc.sync.dma_start(stb[:], b2d.ap().unsqueeze(0))
                nc.vector.tensor_copy(b2rowb[:], stb[:])

                # xs -> transpose -> xsT (fp32) -> scores; also cast to bf16
                xsT = p0.tile([128, ND, SL], F32, tag="xsT", bufs=1)   # 32 KB
                for tt in range(NT):
                    xtm = p0.tile([128, D], F32, tag="xs_tm")
                    nc.sync.dma_start(xtm[:], xs.ap()[tt * 128:(tt + 1) * 128, :])
                    for dt in range(ND):
                        pt = psT.tile([128, 128], F32, tag="ptr")
                        nc.tensor.transpose(
                            pt[:], xtm[:, dt * 128:(dt + 1) * 128], ident_sb[:]
                        )
                        nc.scalar.copy(xsT[:, dt, tt * 128:(tt + 1) * 128], pt[:])

                xsb = p0.tile([128, ND, SL], BF16, tag="xsb", bufs=1)  # 16 KB
                for dt in range(ND):
                    nc.vector.tensor_copy(xsb[:, dt, :], xsT[:, dt, :])

                # scores^T (8, 1024) fp32: accumulate over d tiles
                for tch in range(2):
                    scp = psA.tile([E, 512], F32, tag="sc8", bufs=2)
                    for dt in range(ND):
                        nc.tensor.matmul(
                            scp[:],
                            gw_sb[:, dt, :],
                            xsT[:, dt, tch * 512:(tch + 1) * 512],
                            start=(dt == 0),
                            stop=(dt == ND - 1),
                        )
                    nc.scalar.activation(
                        scT_sl[:, tch * 512:(tch + 1) * 512], scp[:], AF.Tanh
                    )
                nc.vector.tensor_scalar(
                    scT_sl[:], scT_sl[:], 0.5, 0.5, ALU.mult, ALU.add
                )

                # exchange scores: core e receives expert e's full 8192 scores
                nc.sync.dma_start(a2a_in.ap(), scT_sl[:])
                nc.gpsimd.collective_compute(
                    "AllToAll", ALU.bypass, replica_groups=groups,
                    ins=[a2a_in.ap()], outs=[a2a_out.ap()],
                )
                nc.sync.dma_start(
                    sc_own_b[:], a2a_out.ap().flatten().rearrange("(p f) -> p f", p=128)
                )
                nc.sync.dma_start(
                    sc16[:], a2a_out.ap().flatten().rearrange("(f p) -> p f", p=16)
                )

                # capacity predictor on own token slice (overlaps bisection)
                hcap = cappool.tile([128, ND, SL], BF16)    # 16 KB
                for it in range(ND):
                    for tch in range(2):
                        cp = psC.tile([128, 512], F32, tag="caph")
                        for dt in range(ND):
                            nc.tensor.matmul(
                                cp[:],
                                CW1T[:, dt, it * 128:(it + 1) * 128],
                                xsb[:, dt, tch * 512:(tch + 1) * 512],
                                start=(dt == 0),
                                stop=(dt == ND - 1),
                            )
                        nc.scalar.activation(
                            hcap[:, it, tch * 512:(tch + 1) * 512], cp[:],
                            AF.Gelu_apprx_tanh, bias=cb1col[:, it:it + 1],
                        )
                for tch in range(2):
                    lg = psA.tile([E, 512], F32, tag="sc8", bufs=2)
                    for it in range(ND):
                        nc.tensor.matmul(
                            lg[:],
                            cw2b[:, it, :],
                            hcap[:, it, tch * 512:(tch + 1) * 512],
                            start=(it == 0),
                            stop=(it == ND - 1),
                        )
                    nc.scalar.activation(
                        logitsT[:, tch * 512:(tch + 1) * 512], lg[:],
                        AF.Identity, bias=cb2sb[:],
                    )

                # ---- bisection: kth-largest threshold by width halving.
                # invariant count(>= lo) >= K; after N iters lo is within
                # 2^-N of the kth largest score, far inside the boundary gap.
                lo = bis.tile([128, 1], F32, tag="lo")
                nc.vector.memset(lo[:], 0.0)
                for _i in range(BISECT_ITERS):
                    w = 2.0 ** (-(_i + 1))
                    mid = bis.tile([128, 1], F32, tag="mid")
                    nc.vector.tensor_scalar_add(mid[:], lo[:], w)
                    cmp = bis.tile([128, 64], F32, tag="cmp")
                    cntp = bis.tile([128, 1], F32, tag="cntp")
                    nc.vector.tensor_scalar(
                        cmp[:], sc_own_b[:], mid[:], None, ALU.is_ge,
                        ALU.add, accum_out=cntp[:],
                    )
                    cps = psA.tile([128, 1], F32, tag="cnt", bufs=1)
                    nc.tensor.matmul(cps[:], ones_sb[:], cntp[:])
                    gem = bis.tile([128, 1], I8, tag="gem")
                    nc.vector.tensor_scalar(gem[:], cps[:], float(K) - 0.5, None, ALU.is_ge)
                    nc.vector.copy_predicated(lo[:], gem[:], mid[:])

                # share thresholds (for the BCE mask)
                nc.sync.dma_start(tau_in.ap(), lo[0:1, 0:1])
                nc.gpsimd.collective_compute(
                    "AllGather", ALU.bypass, replica_groups=groups,
                    ins=[tau_in.ap()], outs=[tau_out.ap()],
                )
                nc.sync.dma_start(tau_sb[:], tau_out.ap())

                # ---- compaction: selected token ids + their scores
                mask16 = bis.tile([16, 512], I8, tag="mask16")
                nc.vector.tensor_scalar(mask16[:], sc16[:], lo[0:16, :], None, ALU.is_ge)
                midx = bis.tile([16, 512], I16, tag="midx")
                nc.vector.memset(midx[:], -1)
                nc.vector.copy_predicated(midx[:], mask16[:], iota_sb[:])
                smask = bis.tile([16, 512], F32, tag="smask")
                nc.vector.memset(smask[:], -1.0)
                nc.vector.copy_predicated(smask[:], mask16[:], sc16[:])

                lib_sg = nc.gpsimd.load_library(library_config.sparse_gather)
                idx16f = bis.tile([16, 64], F32, tag="idx16f")
                s16 = bis.tile([16, 64], F32, tag="s16")
                nf1 = bis.tile([1, 1], U32, tag="nf1")
                nf2 = bis.tile([1, 1], U32, tag="nf2")
                sg1 = nc.gpsimd.sparse_gather(idx16f[:], midx[:], num_found=nf1[:])
                sg2 = nc.gpsimd.sparse_gather(s16[:], smask[:], num_found=nf2[:])
                _dep(sg1, lib_sg)
                _dep(sg2, sg1)

                idx16 = bis.tile([16, 64], I16, tag="idx16")
                nc.vector.tensor_copy(idx16[:], idx16f[:])
                # bounce through dram: wrapped-16 -> linear, then replicate/load
                nc.sync.dma_start(
                    idx_d.ap().rearrange("(f p) -> p f", p=16), idx16[:]
                )
                nc.sync.dma_start(
                    sel_d.ap().rearrange("(f p) -> p f", p=16), s16[:]
                )
                for a in range(8):
                    nc.sync.dma_start(
                        idx128[16 * a:16 * (a + 1), :],
                        idx_d.ap().rearrange("(f p) -> p f", p=16),
                    )
                s_row = bis.tile([1, SL], F32, tag="s_row")
                nc.sync.dma_start(s_row[:], sel_d.ap().unsqueeze(0))
                nc.vector.tensor_copy(s_rowb[:], s_row[:])
                for tch in range(2):
                    bcp = psC.tile([128, 512], F32, tag="caph")
                    nc.tensor.matmul(
                        bcp[:], ones_sb[0:1, :],
                        s_row[0:1, tch * 512:(tch + 1) * 512],
                    )
                    nc.scalar.copy(s_bc[:, tch * 512:(tch + 1) * 512], bcp[:])

                # ---- BCE loss pieces (token slice x all experts)
                maskT = bis.tile([E, SL], F32, tag="maskT")
                nc.vector.tensor_scalar(maskT[:], scT_sl[:], tau_sb[:], None, ALU.is_ge)
                nc.vector.tensor_tensor(maskT[:], logitsT[:], maskT[:], ALU.mult)
                mxt = bis.tile([E, SL], F32, tag="mxt")
                nc.vector.tensor_scalar(mxt[:], logitsT[:], 0.0, None, ALU.max)
                tmp = bis.tile([E, SL], F32, tag="tmp")
                nc.scalar.activation(tmp[:], logitsT[:], AF.Abs)
                nc.scalar.activation(tmp[:], tmp[:], AF.Exp, scale=-1.0)
                nc.scalar.activation(tmp[:], tmp[:], AF.Ln, bias=1.0)
                nc.vector.tensor_tensor(mxt[:], mxt[:], tmp[:], ALU.add)
                nc.vector.tensor_tensor(mxt[:], mxt[:], maskT[:], ALU.subtract)
                part = bis.tile([E, 1], F32, tag="part")
                nc.vector.tensor_scalar(
                    tmp[:], mxt[:], 1.0, None, ALU.mult, ALU.add,
                    accum_out=part[:]
                )
                lsp = psA.tile([1, 1], F32, tag="cnt", bufs=1)
                nc.tensor.matmul(lsp[:], ones_sb[0:E, 0:1], part[:])
                lsv = bis.tile([1, 1], F32, tag="lsv")
                nc.scalar.mul(lsv[:], lsp[:], 1.0 / (BS * E))
                nc.sync.dma_start(loss_in.ap(), lsv[:])
                nc.gpsimd.collective_compute(
                    "AllReduce", ALU.add, replica_groups=groups,
                    ins=[loss_in.ap()], outs=[loss_out.ap()],
                )
                nc.sync.dma_start(loss.ap(), loss_out.ap())

                lib_mlp = nc.gpsimd.load_library(library_config.mlp)
                _dep(lib_mlp, sg2)

            # ---------------- phase 0b: W1 load + gather + LN + transpose ----
            w1_cm = tc.tile_pool(name="w1pool", bufs=1)
            w1pool = w1_cm.__enter__()
            W1T = w1pool.tile([128, ND, DD], BF16)          # 64 KB/part

            with (
                tc.tile_pool(name="xgpool", bufs=1) as xgpool,
                tc.tile_pool(name="stageb", bufs=1) as stageb,
                tc.tile_pool(name="lnp", bufs=2) as lnp,
                tc.tile_pool(name="psB", bufs=2, space="PSUM") as psB,
            ):
                for dt in range(ND):
                    st = stageb.tile([128, DD], F32, tag="stb")
                    nc.sync.dma_start(st[:], w1t.ap()[dt * 128:(dt + 1) * 128, :])
                    nc.vector.tensor_copy(W1T[:, dt, :], st[:])

                xg = xgpool.tile([128, 8, D], F32)          # 32 KB
                gat = nc.gpsimd.dma_gather(
                    xg[:], xfull.ap(), idx128[:],
                    num_idxs=K, num_idxs_reg=K, elem_size=D,
                )
                _dep(gat, lib_mlp)

                # ---- layernorm (token-major, exact fp32 stats)
                sx = bis.tile([128, 8], F32, tag="sx")
                sxx = bis.tile([128, 8], F32, tag="sxx")
                for q in range(8):
                    scr = lnp.tile([128, D], F32, tag="lnscr")
                    nc.vector.tensor_scalar(
                        scr[:], xg[:, q, :], 1.0, None, ALU.mult,
                        ALU.add, accum_out=sx[:, q:q + 1],
                    )
                    scr2 = lnp.tile([128, D], F32, tag="lnscr2")
                    nc.scalar.activation(
                        scr2[:], xg[:, q, :], AF.Square,
                        accum_out=sxx[:, q:q + 1],
                    )
                mu = bis.tile([128, 8], F32, tag="mu")
                var = bis.tile([128, 8], F32, tag="var")
                rinv = bis.tile([128, 8], F32, tag="rinv")
                nmur = bis.tile([128, 8], F32, tag="nmur")
                nc.vector.tensor_scalar_mul(mu[:], sx[:], 1.0 / D)
                nc.vector.tensor_scalar_mul(var[:], sxx[:], 1.0 / D)
                nc.vector.tensor_tensor(nmur[:], mu[:], mu[:], ALU.mult)
                nc.vector.tensor_tensor(var[:], var[:], nmur[:], ALU.subtract)
                nc.vector.tensor_scalar_add(var[:], var[:], LN_EPS)
                nc.scalar.sqrt(var[:], var[:])
                nc.vector.reciprocal(rinv[:], var[:])
                nc.vector.tensor_tensor(nmur[:], mu[:], rinv[:], ALU.mult)
                nc.vector.tensor_scalar_mul(nmur[:], nmur[:], -1.0)
                for q in range(8):
                    nc.scalar.activation(
                        xg[:, q, :], xg[:, q, :], AF.Identity,
                        bias=nmur[:, q:q + 1], scale=rinv[:, q:q + 1],
                    )
                    for dt in range(ND):
                        pt2 = psB.tile([128, 128], F32, tag="ptr2")
                        nc.tensor.transpose(
                            pt2[:], xg[:, q, dt * 128:(dt + 1) * 128], ident_sb[:]
                        )
                        nc.scalar.activation(
                            xlnT[:, dt, q * 128:(q + 1) * 128], pt2[:],
                            AF.Identity, bias=bcol[:, dt:dt + 1],
                            scale=gcol[:, dt:dt + 1],
                        )

            bis_cm.__exit__(None, None, None)

            # ---------------- phase 1: fc1 for all tokens ----------------
            hs_cm = tc.tile_pool(name="hspool", bufs=1, side="right")
            hspool = hs_cm.__enter__()
            hs = hspool.tile([128, NF, K], BF16)            # 64 KB/part

            with (
                tc.tile_pool(name="psH", bufs=3, space="PSUM") as psH,
                tc.tile_pool(name="hsg", bufs=3) as hsgp,
            ):
                for c in range(2):
                    for ft in range(NF):
                        hp = psH.tile([128, 512], F32, tag="hp")
                        for dt in range(ND):
                            nc.tensor.matmul(
                                hp[:],
                                W1T[:, dt, ft * 128:(ft + 1) * 128],
                                xlnT[:, dt, c * 512:(c + 1) * 512],
                                start=(dt == 0),
                                stop=(dt == ND - 1),
                            )
                        hg = hsgp.tile([128, 512], BF16, tag="hg")
                        nc.scalar.activation(
                            hg[:], hp[:], AF.Gelu_apprx_tanh,
                            bias=b1col[:, ft:ft + 1],
                        )
                        nc.vector.tensor_tensor(
                            hs[:, ft, c * 512:(c + 1) * 512], hg[:],
                            s_bc[:, c * 512:(c + 1) * 512], ALU.mult,
                        )

            w1_cm.__exit__(None, None, None)
            xln_cm.__exit__(None, None, None)

            # ------------- phase 2: fc2 + scatter-add -------------
            with (
                tc.tile_pool(name="w2pool", bufs=1) as w2pool,
                tc.tile_pool(name="stage2", bufs=2) as stage2,
                tc.tile_pool(name="ypool", bufs=1) as ypool,
                tc.tile_pool(name="psY", bufs=1, space="PSUM") as psY,
            ):
                W2T = w2pool.tile([128, NF, D], BF16)       # 64 KB/part
                for ft in range(NF):
                    st2 = stage2.tile([128, D], F32, tag="w2stage")
                    nc.sync.dma_start(st2[:], w2t.ap()[ft * 128:(ft + 1) * 128, :])
                    nc.vector.tensor_copy(W2T[:, ft, :], st2[:])

                prev_scat = lib_mlp
                for c in range(2):
                    yps = [
                        psY.tile([128, 512], F32, tag=f"yp{g}", bufs=1,
                                 name=f"ypt{c}_{g}")
                        for g in range(8)
                    ]
                    for ft in range(NF):
                        for tt in range(4):
                            for dc in range(2):
                                nc.tensor.matmul(
                                    yps[tt * 2 + dc][:],
                                    hs[:, ft, c * 512 + tt * 128:c * 512 + (tt + 1) * 128],
                                    W2T[:, ft, dc * 512:(dc + 1) * 512],
                                    start=(ft == 0),
                                    stop=False,
                                )
                    ysb = ypool.tile([128, 4, D], F32, tag="ysb")
                    for tt in range(4):
                        for dc in range(2):
                            nc.tensor.matmul(
                                yps[tt * 2 + dc][:],
                                s_rowb[0:1, c * 512 + tt * 128:c * 512 + (tt + 1) * 128],
                                b2rowb[0:1, dc * 512:(dc + 1) * 512],
                                start=False,
                                stop=True,
                            )
                            nc.vector.tensor_copy(
                                ysb[:, tt, dc * 512:(dc + 1) * 512],
                                yps[tt * 2 + dc][:],
                            )
                    scat = nc.gpsimd.dma_scatter_add(
                        delta.ap(), ysb[:],
                        idx128[:, c * 32:(c + 1) * 32],
                        num_idxs=512, num_idxs_reg=512, elem_size=D,
                    )
                    _dep(scat, prev_scat)
                    prev_scat = scat

            hs_cm.__exit__(None, None, None)

            # ---------------- phase 3: combine ----------------
            nc.gpsimd.collective_compute(
                "ReduceScatter", ALU.add, replica_groups=groups,
                ins=[delta.ap()], outs=[delta_rs.ap()],
            )
            with tc.tile_pool(name="fin", bufs=3) as fin:
                for tt in range(NT):
                    dl = fin.tile([128, D], F32, tag="dl")
                    nc.sync.dma_start(dl[:], delta_rs.ap()[tt * 128:(tt + 1) * 128, :])
                    xr = fin.tile([128, D], F32, tag="xr")
                    nc.sync.dma_start(xr[:], xs.ap()[tt * 128:(tt + 1) * 128, :])
                    nc.vector.tensor_tensor(dl[:], dl[:], xr[:], ALU.add)
                    nc.sync.dma_start(out_slice.ap()[tt * 128:(tt + 1) * 128, :], dl[:])

    nc.compile()
    return nc


def make_in_maps(inputs):
    """Shard the full inputs into 8 per-core input maps (host-side data
    movement only: slicing, transposes, trivial constants)."""
    x = np.ascontiguousarray(np.asarray(inputs["x"], dtype=np.float32)).reshape(BS, D)
    gate_w = np.asarray(inputs["gate_w"], dtype=np.float32)
    cap_w1 = np.asarray(inputs["cap_w1"], dtype=np.float32)
    cap_b1 = np.asarray(inputs["cap_b1"], dtype=np.float32)
    cap_w2 = np.asarray(inputs["cap_w2"], dtype=np.float32)
    cap_b2 = np.asarray(inputs["cap_b2"], dtype=np.float32)
    norm_w = np.asarray(inputs["norm_w"], dtype=np.float32)
    norm_b = np.asarray(inputs["norm_b"], dtype=np.float32)
    fc1s = np.asarray(inputs["fc1s"], dtype=np.float32)
    b1s = np.asarray(inputs["b1s"], dtype=np.float32)
    fc2s = np.asarray(inputs["fc2s"], dtype=np.float32)
    b2s = np.asarray(inputs["b2s"], dtype=np.float32)

    gwt = np.ascontiguousarray(gate_w.T)
    cw1t = np.ascontiguousarray(cap_w1.T)
    cw2t = np.ascontiguousarray(cap_w2.T)
    ident = np.eye(128, dtype=np.float32)
    ones = np.ones((128, 128), dtype=np.float32)
    # iota16[p, f] = f*16 + p  (token id in wrapped-16 layout)
    iota16 = (np.arange(512, dtype=np.int16)[None, :] * 16
              + np.arange(16, dtype=np.int16)[:, None])
    iota16 = np.ascontiguousarray(iota16)

    in_maps = []
    for r in range(8):
        in_maps.append({
            "xs": np.ascontiguousarray(x[r * SL:(r + 1) * SL, :]),
            "xfull": x,
            "gwt": gwt,
            "w1t": np.ascontiguousarray(fc1s[r].T),
            "w2t": np.ascontiguousarray(fc2s[r].T),
            "b1d": np.ascontiguousarray(b1s[r]),
            "b2d": np.ascontiguousarray(b2s[r]),
            "cw1t": cw1t,
            "cb1d": cap_b1,
            "cw2t": cw2t,
            "cb2d": cap_b2,
            "nwd": norm_w,
            "nbd": norm_b,
            "identd": ident,
            "onesd": ones,
            "iotad": iota16,
        })
    return in_maps


def assemble(results):
    out = np.concatenate(
        [results[r]["out_slice"] for r in range(8)], axis=0
    ).reshape(4, 2048, D)
    cap_loss = np.float32(results[0]["loss"][0, 0])
    return out, cap_loss


def kernel(**inputs):
    if "nc" not in _CACHE:
        _CACHE["nc"] = build_program()
    nc = _CACHE["nc"]
    in_maps = make_in_maps(inputs)
    res = run_bass_kernel_spmd(nc, in_maps, core_ids=list(range(8)))
    return assemble(res.results)


# revision 16
# speedup vs baseline: 1.2759x; 1.2759x over previous
"""DiffMoE MLP (expert-choice routing) Trainium2 kernel.

Model (reference semantics):
  x (4,2048,1024) -> flatten (8192,1024) tokens
  scores = (tanh(x @ gate_w.T)+1)/2            (8192, 8)
  per-expert top-k token selection, k=1024 (expert-choice)
  cap_loss = BCE(cap_mlp(x), keep_mask).mean()
  out = x; out[sel_e] += score * (gelu(LN(x[sel_e]) @ fc1[e].T + b1) @ fc2[e].T + b2)

Distribution (8 NeuronCores):
  - expert-parallel MLP: core e owns expert e's fc1/fc2 (host-sliced inputs)
  - token-parallel scores + capacity predictor: core r owns token slice r
  - AllToAll exchanges scores so core e holds all 8192 scores of expert e
  - top-k via exact fp32 threshold bisection (count >= k), compaction via
    gpsimd sparse_gather, dispatch via dma_gather, combine via local
    dma_scatter_add into a dense buffer + ReduceScatter(add) + residual.

Everything of the reference computation runs on-device. Host only shards
(slices/transposes of weights), concatenates output slices, and supplies
trivial constants (identity/ones/iota).
"""

import numpy as np

import concourse.bass as bass
import concourse.tile as tile
import concourse.mybir as mybir
from concourse import bacc
from concourse import library_config
from concourse.bass_utils import run_bass_kernel_spmd

F32 = mybir.dt.float32
BF16 = mybir.dt.bfloat16
I16 = mybir.dt.int16
I8 = mybir.dt.int8
U32 = mybir.dt.uint32
AF = mybir.ActivationFunctionType
ALU = mybir.AluOpType

# shapes
D = 1024
DD = 4096
E = 8
BS = 8192
SL = 1024          # tokens per core (slice)
K = 1024           # capacity per expert
NT = SL // 128     # 8 token tiles per slice
ND = D // 128      # 8 d tiles
NF = DD // 128     # 32 f tiles
LN_EPS = 1e-5
BISECT_ITERS = 24

_CACHE = {}


def _dep(later, earlier):
    """Order `later` after `earlier` on the same engine stream."""
    tile.add_dep_helper(later.ins, earlier.ins, sync=True)


def build_program():
    nc = bacc.Bacc("TRN2", target_bir_lowering=False, debug=False, num_devices=8,
                   num_swdge_queues=4)

    # ---------------- I/O ----------------
    xs = nc.dram_tensor("xs", [SL, D], F32, kind="ExternalInput")
    xfull = nc.dram_tensor("xfull", [BS, D], F32, kind="ExternalInput")
    gwc = nc.dram_tensor("gwc", [128, ND, E], F32, kind="ExternalInput")
    w1t = nc.dram_tensor("w1t", [D, DD], F32, kind="ExternalInput")
    w2t = nc.dram_tensor("w2t", [DD, D], F32, kind="ExternalInput")
    b1c = nc.dram_tensor("b1c", [128, NF], F32, kind="ExternalInput")
    b2d = nc.dram_tensor("b2d", [D], F32, kind="ExternalInput")
    cw1t = nc.dram_tensor("cw1t", [D, D], F32, kind="ExternalInput")
    cb1c = nc.dram_tensor("cb1c", [128, ND], F32, kind="ExternalInput")
    cw2c = nc.dram_tensor("cw2c", [128, ND, E], F32, kind="ExternalInput")
    cb2d = nc.dram_tensor("cb2d", [E], F32, kind="ExternalInput")
    nwc = nc.dram_tensor("nwc", [128, ND], F32, kind="ExternalInput")
    nbc = nc.dram_tensor("nbc", [128, ND], F32, kind="ExternalInput")
    identd = nc.dram_tensor("identd", [128, 128], F32, kind="ExternalInput")
    onesd = nc.dram_tensor("onesd", [128, 128], F32, kind="ExternalInput")
    iotad = nc.dram_tensor("iotad", [16, 512], I16, kind="ExternalInput")

    out_slice = nc.dram_tensor("out_slice", [SL, D], F32, kind="ExternalOutput")
    loss = nc.dram_tensor("loss", [1, 1], F32, kind="ExternalOutput")

    # internal dram
    a2a_in = nc.dram_tensor("a2a_in", [E, SL], F32)
    a2a_out = nc.dram_tensor("a2a_out", [E, SL], F32)
    tau_in = nc.dram_tensor("tau_in", [1, 1], F32)
    tau_out = nc.dram_tensor("tau_out", [E, 1], F32)
    idx_d = nc.dram_tensor("idx_d", [K], I16)
    sel_d = nc.dram_tensor("sel_d", [K], F32)
    delta = nc.dram_tensor("delta", [BS, D], BF16)
    delta_rs = nc.dram_tensor("delta_rs", [SL, D], BF16)
    loss_in = nc.dram_tensor("loss_in", [1, 1], F32)
    loss_out = nc.dram_tensor("loss_out", [1, 1], F32)

    groups = [list(range(8))]

    with tile.TileContext(nc) as tc:
        with tc.tile_pool(name="smalls", bufs=1) as smalls:
            gw_sb = smalls.tile([128, ND, E], F32)
            b1col = smalls.tile([128, NF], F32)
            gcol = smalls.tile([128, ND], F32)
            bcol = smalls.tile([128, ND], F32)
            cb1col = smalls.tile([128, ND], F32)
            cb2sb = smalls.tile([E, 1], F32)
            b2rowb = smalls.tile([1, D], BF16)
            ident_sb = smalls.tile([128, 128], F32)
            ones_sb = smalls.tile([128, 128], F32)
            iota_sb = smalls.tile([16, 512], I16)
            scT_sl = smalls.tile([E, SL], F32)
            logitsT = smalls.tile([E, SL], F32)
            sc_own_b = smalls.tile([128, 64], F32)
            sc16 = smalls.tile([16, 512], F32)
            tau_sb = smalls.tile([E, 1], F32)
            idx128 = smalls.tile([128, 64], I16)
            s128 = smalls.tile([128, 8], F32)
            b2bc = smalls.tile([128, D], F32)

            # const loads
            nc.sync.dma_start(ident_sb[:], identd[:, :])
            nc.sync.dma_start(ones_sb[:], onesd[:, :])
            nc.sync.dma_start(iota_sb[:], iotad[:, :])
            nc.sync.dma_start(gw_sb[:], gwc[:, :, :])
            nc.sync.dma_start(b1col[:], b1c[:, :])
            nc.sync.dma_start(gcol[:], nwc[:, :])
            nc.sync.dma_start(bcol[:], nbc[:, :])
            nc.sync.dma_start(cb1col[:], cb1c[:, :])
            nc.sync.dma_start(cb2sb[:], cb2d.ap().unsqueeze(1))

            # ---------------- phase 0a: scores + cap mlp + bisect ------------
            bis_cm = tc.tile_pool(name="bis", bufs=1, side="right")
            bis = bis_cm.__enter__()
            xln_cm = tc.tile_pool(name="xlnpool", bufs=1)
            xlnpool = xln_cm.__enter__()
            xlnT = xlnpool.tile([128, ND, K], BF16)

            with (
                tc.tile_pool(name="stage", bufs=2) as stage,
                tc.tile_pool(name="p0", bufs=2) as p0,
                tc.tile_pool(name="cappool", bufs=1) as cappool,
                tc.tile_pool(name="psA", bufs=1, space="PSUM") as psA,
                tc.tile_pool(name="psT", bufs=2, space="PSUM") as psT,
                tc.tile_pool(name="psC", bufs=2, space="PSUM") as psC,
            ):
                # zero the dense combine buffer early (overlaps everything)
                zt = stage.tile([128, 8192], BF16, tag="st", bufs=2)
                nc.vector.memset(zt[:], 0.0)
                for c in range(8):
                    nc.sync.dma_start(
                        delta.ap()[c * 1024:(c + 1) * 1024, :], zt[:]
                    )

                CW1T = cappool.tile([128, ND, D], BF16)     # 16 KB/part
                for dt in range(ND):
                    st = stage.tile([128, D], F32, tag="st", bufs=2)
                    nc.sync.dma_start(st[:], cw1t.ap()[dt * 128:(dt + 1) * 128, :])
                    nc.vector.tensor_copy(CW1T[:, dt, :], st[:])
                cw2b = cappool.tile([128, ND, E], BF16)
                stc = stage.tile([128, ND, E], F32, tag="st", bufs=2)
                nc.sync.dma_start(stc[:], cw2c[:, :, :])
                nc.vector.tensor_copy(cw2b[:], stc[:])
                # b2 broadcast tile: ones(128,1) x b2 via two K=1 matmuls
                stb = stage.tile([1, D], F32, tag="st", bufs=2)
                nc.sync.dma_start(stb[:], b2d.ap().unsqueeze(0))
                for tch in range(2):
                    bp = psC.tile([128, 512], F32, tag="caph")
                    nc.tensor.matmul(
                        bp[:], ones_sb[0:1, :], stb[0:1, tch * 512:(tch + 1) * 512]
                    )
                    nc.scalar.copy(b2bc[:, tch * 512:(tch + 1) * 512], bp[:])

                # xs -> transpose -> xsT (fp32) -> scores; also cast to bf16
                xsT = p0.tile([128, ND, SL], F32, tag="xsT", bufs=1)   # 32 KB
                for tt in range(NT):
                    xtm = p0.tile([128, D], F32, tag="xs_tm")
                    nc.sync.dma_start(xtm[:], xs.ap()[tt * 128:(tt + 1) * 128, :])
                    for dt in range(ND):
                        pt = psT.tile([128, 128], F32, tag="ptr")
                        nc.tensor.transpose(
                            pt[:], xtm[:, dt * 128:(dt + 1) * 128], ident_sb[:]
                        )
                        nc.scalar.copy(xsT[:, dt, tt * 128:(tt + 1) * 128], pt[:])

                xsb = p0.tile([128, ND, SL], BF16, tag="xsb", bufs=1)  # 16 KB
                for dt in range(ND):
                    nc.vector.tensor_copy(xsb[:, dt, :], xsT[:, dt, :])

                # scores^T (8, 1024) fp32: accumulate over d tiles
                for tch in range(2):
                    scp = psA.tile([E, 512], F32, tag="sc8", bufs=2)
                    for dt in range(ND):
                        nc.tensor.matmul(
                            scp[:],
                            gw_sb[:, dt, :],
                            xsT[:, dt, tch * 512:(tch + 1) * 512],
                            start=(dt == 0),
                            stop=(dt == ND - 1),
                        )
                    nc.scalar.activation(
                        scT_sl[:, tch * 512:(tch + 1) * 512], scp[:], AF.Tanh
                    )
                nc.vector.tensor_scalar(
                    scT_sl[:], scT_sl[:], 0.5, 0.5, ALU.mult, ALU.add
                )

                # exchange scores: core e receives expert e's full 8192 scores
                nc.sync.dma_start(a2a_in.ap(), scT_sl[:])
                nc.gpsimd.collective_compute(
                    "AllToAll", ALU.bypass, replica_groups=groups,
                    ins=[a2a_in.ap()], outs=[a2a_out.ap()],
                )
                nc.sync.dma_start(
                    sc_own_b[:], a2a_out.ap().flatten().rearrange("(p f) -> p f", p=128)
                )
                nc.sync.dma_start(
                    sc16[:], a2a_out.ap().flatten().rearrange("(p f) -> p f", p=16)
                )

                # capacity predictor on own token slice (overlaps bisection)
                hcap = cappool.tile([128, ND, SL], BF16)    # 16 KB
                for it in range(ND):
                    for tch in range(2):
                        cp = psC.tile([128, 512], F32, tag="caph")
                        for dt in range(ND):
                            nc.tensor.matmul(
                                cp[:],
                                CW1T[:, dt, it * 128:(it + 1) * 128],
                                xsb[:, dt, tch * 512:(tch + 1) * 512],
                                start=(dt == 0),
                                stop=(dt == ND - 1),
                            )
                        nc.scalar.activation(
                            hcap[:, it, tch * 512:(tch + 1) * 512], cp[:],
                            AF.Gelu_apprx_tanh, bias=cb1col[:, it:it + 1],
                        )
                for tch in range(2):
                    lg = psA.tile([E, 512], F32, tag="sc8", bufs=2)
                    for it in range(ND):
                        nc.tensor.matmul(
                            lg[:],
                            cw2b[:, it, :],
                            hcap[:, it, tch * 512:(tch + 1) * 512],
                            start=(it == 0),
                            stop=(it == ND - 1),
                        )
                    nc.scalar.activation(
                        logitsT[:, tch * 512:(tch + 1) * 512], lg[:],
                        AF.Identity, bias=cb2sb[:],
                    )

                # ---- bisection: kth-largest threshold by width halving.
                # invariant count(>= lo) >= K; after N iters lo is within
                # 2^-N of the kth largest score, far inside the boundary gap.
                lo = bis.tile([128, 1], F32, tag="lo")
                nc.vector.memset(lo[:], 0.0)
                for _i in range(BISECT_ITERS):
                    w = 2.0 ** (-(_i + 1))
                    mid = bis.tile([128, 1], F32, tag="mid")
                    nc.vector.tensor_scalar_add(mid[:], lo[:], w)
                    cmp = bis.tile([128, 64], F32, tag="cmp")
                    cntp = bis.tile([128, 1], F32, tag="cntp")
                    nc.vector.tensor_scalar(
                        cmp[:], sc_own_b[:], mid[:], None, ALU.is_ge,
                        ALU.add, accum_out=cntp[:],
                    )
                    cps = psA.tile([128, 1], F32, tag="cnt", bufs=1)
                    nc.tensor.matmul(cps[:], ones_sb[:], cntp[:])
                    gem = bis.tile([128, 1], I8, tag="gem")
                    nc.vector.tensor_scalar(gem[:], cps[:], float(K) - 0.5, None, ALU.is_ge)
                    nc.vector.copy_predicated(lo[:], gem[:], mid[:])

                # share thresholds (for the BCE mask)
                nc.sync.dma_start(tau_in.ap(), lo[0:1, 0:1])
                nc.gpsimd.collective_compute(
                    "AllGather", ALU.bypass, replica_groups=groups,
                    ins=[tau_in.ap()], outs=[tau_out.ap()],
                )
                nc.sync.dma_start(tau_sb[:], tau_out.ap())

                # ---- compaction: selected token ids + their scores
                mask16 = bis.tile([16, 512], I8, tag="mask16")
                nc.vector.tensor_scalar(mask16[:], sc16[:], lo[0:16, :], None, ALU.is_ge)
                midx = bis.tile([16, 512], I16, tag="midx")
                nc.vector.memset(midx[:], -1)
                nc.vector.copy_predicated(midx[:], mask16[:], iota_sb[:])
                smask = bis.tile([16, 512], F32, tag="smask")
                nc.vector.memset(smask[:], -1.0)
                nc.vector.copy_predicated(smask[:], mask16[:], sc16[:])

                lib_sg = nc.gpsimd.load_library(library_config.sparse_gather)
                idx16f = bis.tile([16, 64], F32, tag="idx16f")
                s16 = bis.tile([16, 64], F32, tag="s16")
                nf1 = bis.tile([1, 1], U32, tag="nf1")
                nf2 = bis.tile([1, 1], U32, tag="nf2")
                sg1 = nc.gpsimd.sparse_gather(idx16f[:], midx[:], num_found=nf1[:])
                sg2 = nc.gpsimd.sparse_gather(s16[:], smask[:], num_found=nf2[:])
                _dep(sg1, lib_sg)
                _dep(sg2, sg1)

                idx16 = bis.tile([16, 64], I16, tag="idx16")
                nc.vector.tensor_copy(idx16[:], idx16f[:])
                # bounce through dram (linear layouts, few descriptors)
                nc.sync.dma_start(
                    idx_d.ap().rearrange("(p f) -> p f", p=16), idx16[:]
                )
                nc.sync.dma_start(
                    sel_d.ap().rearrange("(p f) -> p f", p=16), s16[:]
                )
                for a in range(8):
                    nc.sync.dma_start(
                        idx128[16 * a:16 * (a + 1), :],
                        idx_d.ap().rearrange("(p f) -> p f", p=16),
                    )
                # s128[p, q] = score of selection slot s = q*128+p, which
                # lives at sel_d[(s%16)*64 + s//16]
                nc.sync.dma_start(
                    s128[:],
                    sel_d.ap().rearrange("(b q a) -> a b q", b=16, q=8, a=8),
                )

                # ---- BCE loss pieces (token slice x all experts)
                maskT = bis.tile([E, SL], F32, tag="maskT")
                nc.vector.tensor_scalar(maskT[:], scT_sl[:], tau_sb[:], None, ALU.is_ge)
                nc.vector.tensor_tensor(maskT[:], logitsT[:], maskT[:], ALU.mult)
                mxt = bis.tile([E, SL], F32, tag="mxt")
                nc.vector.tensor_scalar(mxt[:], logitsT[:], 0.0, None, ALU.max)
                tmp = bis.tile([E, SL], F32, tag="tmp")
                nc.scalar.activation(tmp[:], logitsT[:], AF.Abs)
                nc.scalar.activation(tmp[:], tmp[:], AF.Exp, scale=-1.0)
                nc.scalar.activation(tmp[:], tmp[:], AF.Ln, bias=1.0)
                nc.vector.tensor_tensor(mxt[:], mxt[:], tmp[:], ALU.add)
                nc.vector.tensor_tensor(mxt[:], mxt[:], maskT[:], ALU.subtract)
                part = bis.tile([E, 1], F32, tag="part")
                nc.vector.tensor_scalar(
                    tmp[:], mxt[:], 1.0, None, ALU.mult, ALU.add,
                    accum_out=part[:]
                )
                lsp = psA.tile([1, 1], F32, tag="cnt", bufs=1)
                nc.tensor.matmul(lsp[:], ones_sb[0:E, 0:1], part[:])
                lsv = bis.tile([1, 1], F32, tag="lsv")
                nc.scalar.mul(lsv[:], lsp[:], 1.0 / (BS * E))
                nc.sync.dma_start(loss_in.ap(), lsv[:])
                nc.gpsimd.collective_compute(
                    "AllReduce", ALU.add, replica_groups=groups,
                    ins=[loss_in.ap()], outs=[loss_out.ap()],
                )
                nc.sync.dma_start(loss.ap(), loss_out.ap())

                lib_mlp = nc.gpsimd.load_library(library_config.mlp)
                _dep(lib_mlp, sg2)

            # ---------------- phase 0b: W1 load + gather + LN + transpose ----
            w1_cm = tc.tile_pool(name="w1pool", bufs=1)
            w1pool = w1_cm.__enter__()
            W1T = w1pool.tile([128, ND, DD], BF16)          # 64 KB/part

            with (
                tc.tile_pool(name="xgpool", bufs=1) as xgpool,
                tc.tile_pool(name="stageb", bufs=1) as stageb,
                tc.tile_pool(name="lnp", bufs=2) as lnp,
                tc.tile_pool(name="psB", bufs=2, space="PSUM") as psB,
            ):
                for dt in range(ND):
                    st = stageb.tile([128, DD], F32, tag="stb")
                    nc.sync.dma_start(st[:], w1t.ap()[dt * 128:(dt + 1) * 128, :])
                    nc.vector.tensor_copy(W1T[:, dt, :], st[:])

                xg = xgpool.tile([128, 8, D], F32)          # 32 KB
                prev_g = lib_mlp
                for a in range(4):
                    gat = nc.gpsimd.dma_gather(
                        xg[:, 2 * a:2 * a + 2, :], xfull.ap(),
                        idx128[:, 16 * a:16 * (a + 1)],
                        num_idxs=256, num_idxs_reg=256, elem_size=D,
                        queue_num=a,
                    )
                    _dep(gat, prev_g)
                    prev_g = gat

                # ---- layernorm (token-major, exact fp32 stats)
                sx = bis.tile([128, 8], F32, tag="sx")
                sxx = bis.tile([128, 8], F32, tag="sxx")
                for q in range(8):
                    scr = lnp.tile([128, D], F32, tag="lnscr")
                    nc.vector.tensor_scalar(
                        scr[:], xg[:, q, :], 1.0, None, ALU.mult,
                        ALU.add, accum_out=sx[:, q:q + 1],
                    )
                    scr2 = lnp.tile([128, D], F32, tag="lnscr2")
                    nc.scalar.activation(
                        scr2[:], xg[:, q, :], AF.Square,
                        accum_out=sxx[:, q:q + 1],
                    )
                mu = bis.tile([128, 8], F32, tag="mu")
                var = bis.tile([128, 8], F32, tag="var")
                rinv = bis.tile([128, 8], F32, tag="rinv")
                nmur = bis.tile([128, 8], F32, tag="nmur")
                nc.vector.tensor_scalar_mul(mu[:], sx[:], 1.0 / D)
                nc.vector.tensor_scalar_mul(var[:], sxx[:], 1.0 / D)
                nc.vector.tensor_tensor(nmur[:], mu[:], mu[:], ALU.mult)
                nc.vector.tensor_tensor(var[:], var[:], nmur[:], ALU.subtract)
                nc.vector.tensor_scalar_add(var[:], var[:], LN_EPS)
                nc.scalar.sqrt(var[:], var[:])
                nc.vector.reciprocal(rinv[:], var[:])
                nc.vector.tensor_tensor(nmur[:], mu[:], rinv[:], ALU.mult)
                nc.vector.tensor_scalar_mul(nmur[:], nmur[:], -1.0)
                for q in range(8):
                    nc.scalar.activation(
                        xg[:, q, :], xg[:, q, :], AF.Identity,
                        bias=nmur[:, q:q + 1], scale=rinv[:, q:q + 1],
                    )
                    for dt in range(ND):
                        pt2 = psB.tile([128, 128], F32, tag="ptr2")
                        nc.tensor.transpose(
                            pt2[:], xg[:, q, dt * 128:(dt + 1) * 128], ident_sb[:]
                        )
                        nc.scalar.activation(
                            xlnT[:, dt, q * 128:(q + 1) * 128], pt2[:],
                            AF.Identity, bias=bcol[:, dt:dt + 1],
                            scale=gcol[:, dt:dt + 1],
                        )

            bis_cm.__exit__(None, None, None)

            # ---------------- phase 1: fc1 for all tokens ----------------
            hs_cm = tc.tile_pool(name="hspool", bufs=1, side="right")
            hspool = hs_cm.__enter__()
            hs = hspool.tile([128, NF, K], BF16)            # 64 KB/part

            with (
                tc.tile_pool(name="psH", bufs=3, space="PSUM") as psH,
                tc.tile_pool(name="hsg", bufs=3) as hsgp,
            ):
                for c in range(2):
                    for ft in range(NF):
                        hp = psH.tile([128, 512], F32, tag="hp")
                        for dt in range(ND):
                            nc.tensor.matmul(
                                hp[:],
                                W1T[:, dt, ft * 128:(ft + 1) * 128],
                                xlnT[:, dt, c * 512:(c + 1) * 512],
                                start=(dt == 0),
                                stop=(dt == ND - 1),
                            )
                        nc.scalar.activation(
                            hs[:, ft, c * 512:(c + 1) * 512], hp[:],
                            AF.Gelu_apprx_tanh, bias=b1col[:, ft:ft + 1],
                        )

            w1_cm.__exit__(None, None, None)
            xln_cm.__exit__(None, None, None)

            # ------------- phase 2: fc2 + scatter-add -------------
            with (
                tc.tile_pool(name="w2pool", bufs=1) as w2pool,
                tc.tile_pool(name="stage2", bufs=2) as stage2,
                tc.tile_pool(name="ypool", bufs=1) as ypool,
                tc.tile_pool(name="psY", bufs=1, space="PSUM") as psY,
            ):
                W2T = w2pool.tile([128, NF, D], BF16)       # 64 KB/part
                for ft in range(NF):
                    st2 = stage2.tile([128, D], F32, tag="w2stage")
                    nc.sync.dma_start(st2[:], w2t.ap()[ft * 128:(ft + 1) * 128, :])
                    nc.vector.tensor_copy(W2T[:, ft, :], st2[:])

                prev_scat = lib_mlp
                for c in range(2):
                    yps = [
                        psY.tile([128, 512], F32, tag=f"yp{g}", bufs=1,
                                 name=f"ypt{c}_{g}")
                        for g in range(8)
                    ]
                    for ft in range(NF):
                        for tt in range(4):
                            for dc in range(2):
                                nc.tensor.matmul(
                                    yps[tt * 2 + dc][:],
                                    hs[:, ft, c * 512 + tt * 128:c * 512 + (tt + 1) * 128],
                                    W2T[:, ft, dc * 512:(dc + 1) * 512],
                                    start=(ft == 0),
                                    stop=(ft == NF - 1),
                                )
                    ysb = ypool.tile([128, 4, D], BF16, tag="ysb")
                    ytmp = ypool.tile([128, 512], F32, tag="ytmp", bufs=2)
                    for tt in range(4):
                        for dc in range(2):
                            nc.vector.tensor_tensor(
                                ytmp[:], yps[tt * 2 + dc][:],
                                b2bc[:, dc * 512:(dc + 1) * 512], ALU.add,
                            )
                            nc.vector.tensor_scalar(
                                ysb[:, tt, dc * 512:(dc + 1) * 512],
                                ytmp[:], s128[:, c * 4 + tt:c * 4 + tt + 1],
                                None, ALU.mult,
                            )
                    for a in range(4):
                        scat = nc.gpsimd.dma_scatter_add(
                            delta.ap(), ysb[:, a:a + 1, :],
                            idx128[:, c * 32 + 8 * a:c * 32 + 8 * (a + 1)],
                            num_idxs=128, num_idxs_reg=128, elem_size=D,
                            queue_num=a,
                        )
                        _dep(scat, prev_scat)
                        prev_scat = scat

            hs_cm.__exit__(None, None, None)

            # ---------------- phase 3: combine ----------------
            nc.gpsimd.collective_compute(
                "ReduceScatter", ALU.add, replica_groups=groups,
                ins=[delta.ap()], outs=[delta_rs.ap()],
            )
            with tc.tile_pool(name="fin", bufs=3) as fin:
                for tt in range(NT):
                    dl = fin.tile([128, D], BF16, tag="dl")
                    nc.sync.dma_start(dl[:], delta_rs.ap()[tt * 128:(tt + 1) * 128, :])
                    xr = fin.tile([128, D], F32, tag="xr")
                    nc.sync.dma_start(xr[:], xs.ap()[tt * 128:(tt + 1) * 128, :])
                    ov = fin.tile([128, D], F32, tag="ov")
                    nc.vector.tensor_tensor(ov[:], dl[:], xr[:], ALU.add)
                    nc.sync.dma_start(out_slice.ap()[tt * 128:(tt + 1) * 128, :], ov[:])

    nc.compile()
    return nc


def make_in_maps(inputs):
    """Shard the full inputs into 8 per-core input maps (host-side data
    movement only: slicing, transposes, trivial constants)."""
    x = np.ascontiguousarray(np.asarray(inputs["x"], dtype=np.float32)).reshape(BS, D)
    gate_w = np.asarray(inputs["gate_w"], dtype=np.float32)
    cap_w1 = np.asarray(inputs["cap_w1"], dtype=np.float32)
    cap_b1 = np.asarray(inputs["cap_b1"], dtype=np.float32)
    cap_w2 = np.asarray(inputs["cap_w2"], dtype=np.float32)
    cap_b2 = np.asarray(inputs["cap_b2"], dtype=np.float32)
    norm_w = np.asarray(inputs["norm_w"], dtype=np.float32)
    norm_b = np.asarray(inputs["norm_b"], dtype=np.float32)
    fc1s = np.asarray(inputs["fc1s"], dtype=np.float32)
    b1s = np.asarray(inputs["b1s"], dtype=np.float32)
    fc2s = np.asarray(inputs["fc2s"], dtype=np.float32)
    b2s = np.asarray(inputs["b2s"], dtype=np.float32)

    cw1t = np.ascontiguousarray(cap_w1.T)
    # [p, dt, e] layouts for per-partition stationary tiles
    gwc = np.ascontiguousarray(gate_w.T.reshape(8, 128, E).transpose(1, 0, 2))
    cw2c = np.ascontiguousarray(cap_w2.T.reshape(8, 128, E).transpose(1, 0, 2))
    nwc = np.ascontiguousarray(norm_w.reshape(8, 128).T)
    nbc = np.ascontiguousarray(norm_b.reshape(8, 128).T)
    cb1c = np.ascontiguousarray(cap_b1.reshape(8, 128).T)
    ident = np.eye(128, dtype=np.float32)
    ones = np.ones((128, 128), dtype=np.float32)
    # iota16[p, f] = p*512 + f  (token id at position (p, f) of the
    # linearly-loaded sc16 tile)
    iota16 = (np.arange(16, dtype=np.int16)[:, None] * 512
              + np.arange(512, dtype=np.int16)[None, :])
    iota16 = np.ascontiguousarray(iota16)

    in_maps = []
    for r in range(8):
        in_maps.append({
            "xs": np.ascontiguousarray(x[r * SL:(r + 1) * SL, :]),
            "xfull": x,
            "gwc": gwc,
            "w1t": np.ascontiguousarray(fc1s[r].T),
            "w2t": np.ascontiguousarray(fc2s[r].T),
            "b1c": np.ascontiguousarray(b1s[r].reshape(32, 128).T),
            "b2d": np.ascontiguousarray(b2s[r]),
            "cw1t": cw1t,
            "cb1c": cb1c,
            "cw2c": cw2c,
            "cb2d": cap_b2,
            "nwc": nwc,
            "nbc": nbc,
            "identd": ident,
            "onesd": ones,
            "iotad": iota16,
        })
    return in_maps


def assemble(results):
    out = np.concatenate(
        [results[r]["out_slice"] for r in range(8)], axis=0
    ).reshape(4, 2048, D)
    cap_loss = np.float32(results[0]["loss"][0, 0])
    return out, cap_loss


def kernel(**inputs):
    if "nc" not in _CACHE:
        _CACHE["nc"] = build_program()
    nc = _CACHE["nc"]
    in_maps = make_in_maps(inputs)
    res = run_bass_kernel_spmd(nc, in_maps, core_ids=list(range(8)))
    return assemble(res.results)


# revision 17
# speedup vs baseline: 1.3428x; 1.0524x over previous
"""DiffMoE MLP (expert-choice routing) Trainium2 kernel.

Model (reference semantics):
  x (4,2048,1024) -> flatten (8192,1024) tokens
  scores = (tanh(x @ gate_w.T)+1)/2            (8192, 8)
  per-expert top-k token selection, k=1024 (expert-choice)
  cap_loss = BCE(cap_mlp(x), keep_mask).mean()
  out = x; out[sel_e] += score * (gelu(LN(x[sel_e]) @ fc1[e].T + b1) @ fc2[e].T + b2)

Distribution (8 NeuronCores):
  - expert-parallel MLP: core e owns expert e's fc1/fc2 (host-sliced inputs)
  - token-parallel scores + capacity predictor: core r owns token slice r
  - AllToAll exchanges scores so core e holds all 8192 scores of expert e
  - top-k via exact fp32 threshold bisection (count >= k), compaction via
    gpsimd sparse_gather, dispatch via dma_gather, combine via local
    dma_scatter_add into a dense buffer + ReduceScatter(add) + residual.

Everything of the reference computation runs on-device. Host only shards
(slices/transposes of weights), concatenates output slices, and supplies
trivial constants (identity/ones/iota).
"""

import numpy as np

import concourse.bass as bass
import concourse.tile as tile
import concourse.mybir as mybir
from concourse import bacc
from concourse import library_config
from concourse.bass_utils import run_bass_kernel_spmd

F32 = mybir.dt.float32
BF16 = mybir.dt.bfloat16
I16 = mybir.dt.int16
I8 = mybir.dt.int8
U32 = mybir.dt.uint32
AF = mybir.ActivationFunctionType
ALU = mybir.AluOpType

# shapes
D = 1024
DD = 4096
E = 8
BS = 8192
SL = 1024          # tokens per core (slice)
K = 1024           # capacity per expert
NT = SL // 128     # 8 token tiles per slice
ND = D // 128      # 8 d tiles
NF = DD // 128     # 32 f tiles
LN_EPS = 1e-5
BISECT_ITERS = 16

_CACHE = {}


def _dep(later, earlier):
    """Order `later` after `earlier` on the same engine stream."""
    tile.add_dep_helper(later.ins, earlier.ins, sync=True)


def build_program():
    nc = bacc.Bacc("TRN2", target_bir_lowering=False, debug=False, num_devices=8,
                   num_swdge_queues=4)

    # ---------------- I/O ----------------
    xs = nc.dram_tensor("xs", [SL, D], F32, kind="ExternalInput")
    xfull = nc.dram_tensor("xfull", [BS, D], F32, kind="ExternalInput")
    gwc = nc.dram_tensor("gwc", [128, ND, E], F32, kind="ExternalInput")
    w1t = nc.dram_tensor("w1t", [D, DD], F32, kind="ExternalInput")
    w2t = nc.dram_tensor("w2t", [DD, D], F32, kind="ExternalInput")
    b1c = nc.dram_tensor("b1c", [128, NF], F32, kind="ExternalInput")
    b2d = nc.dram_tensor("b2d", [D], F32, kind="ExternalInput")
    cw1t = nc.dram_tensor("cw1t", [D, D], F32, kind="ExternalInput")
    cb1c = nc.dram_tensor("cb1c", [128, ND], F32, kind="ExternalInput")
    cw2c = nc.dram_tensor("cw2c", [128, ND, E], F32, kind="ExternalInput")
    cb2d = nc.dram_tensor("cb2d", [E], F32, kind="ExternalInput")
    nwc = nc.dram_tensor("nwc", [128, ND], F32, kind="ExternalInput")
    nbc = nc.dram_tensor("nbc", [128, ND], F32, kind="ExternalInput")
    identd = nc.dram_tensor("identd", [128, 128], F32, kind="ExternalInput")
    onesd = nc.dram_tensor("onesd", [128, 128], F32, kind="ExternalInput")
    iotad = nc.dram_tensor("iotad", [16, 512], I16, kind="ExternalInput")

    out_slice = nc.dram_tensor("out_slice", [SL, D], F32, kind="ExternalOutput")
    loss = nc.dram_tensor("loss", [1, 1], F32, kind="ExternalOutput")

    # internal dram
    a2a_in = nc.dram_tensor("a2a_in", [E, SL], F32)
    a2a_out = nc.dram_tensor("a2a_out", [E, SL], F32)
    tau_in = nc.dram_tensor("tau_in", [1, 1], F32)
    tau_out = nc.dram_tensor("tau_out", [E, 1], F32)
    sel_d = nc.dram_tensor("sel_d", [K], F32)
    delta_a = nc.dram_tensor("delta_a", [BS, D // 2], BF16)
    delta_b = nc.dram_tensor("delta_b", [BS, D // 2], BF16)
    delta_rs_a = nc.dram_tensor("delta_rs_a", [SL, D // 2], BF16)
    delta_rs_b = nc.dram_tensor("delta_rs_b", [SL, D // 2], BF16)
    loss_in = nc.dram_tensor("loss_in", [1, 1], F32)
    loss_out = nc.dram_tensor("loss_out", [1, 1], F32)

    groups = [list(range(8))]

    with tile.TileContext(nc) as tc:
        with tc.tile_pool(name="smalls", bufs=1) as smalls:
            gw_sb = smalls.tile([128, ND, E], F32)
            b1col = smalls.tile([128, NF], F32)
            gcol = smalls.tile([128, ND], F32)
            bcol = smalls.tile([128, ND], F32)
            cb1col = smalls.tile([128, ND], F32)
            cb2sb = smalls.tile([E, 1], F32)
            b2rowb = smalls.tile([1, D], BF16)
            ident_sb = smalls.tile([128, 128], F32)
            ones_sb = smalls.tile([128, 128], F32)
            iota_sb = smalls.tile([16, 512], I16)
            scT_sl = smalls.tile([E, SL], F32)
            logitsT = smalls.tile([E, SL], F32)
            sc_own_b = smalls.tile([128, 64], F32)
            sc16 = smalls.tile([16, 512], F32)
            tau_sb = smalls.tile([E, 1], F32)
            idx128 = smalls.tile([128, 64], I16)
            s128 = smalls.tile([128, 8], F32)
            b2bc = smalls.tile([128, D], F32)

            # const loads
            nc.sync.dma_start(ident_sb[:], identd[:, :])
            nc.sync.dma_start(ones_sb[:], onesd[:, :])
            nc.sync.dma_start(iota_sb[:], iotad[:, :])
            nc.sync.dma_start(gw_sb[:], gwc[:, :, :])
            nc.sync.dma_start(b1col[:], b1c[:, :])
            nc.sync.dma_start(gcol[:], nwc[:, :])
            nc.sync.dma_start(bcol[:], nbc[:, :])
            nc.sync.dma_start(cb1col[:], cb1c[:, :])
            nc.sync.dma_start(cb2sb[:], cb2d.ap().unsqueeze(1))

            # ---------------- phase 0a: scores + cap mlp + bisect ------------
            bis_cm = tc.tile_pool(name="bis", bufs=1, side="right")
            bis = bis_cm.__enter__()
            xln_cm = tc.tile_pool(name="xlnpool", bufs=1)
            xlnpool = xln_cm.__enter__()
            xlnT = xlnpool.tile([128, ND, K], BF16)

            with (
                tc.tile_pool(name="stage", bufs=2) as stage,
                tc.tile_pool(name="p0", bufs=2) as p0,
                tc.tile_pool(name="cappool", bufs=1) as cappool,
                tc.tile_pool(name="psA", bufs=1, space="PSUM") as psA,
                tc.tile_pool(name="psT", bufs=2, space="PSUM") as psT,
                tc.tile_pool(name="psC", bufs=2, space="PSUM") as psC,
            ):
                # zero the dense combine buffer early (overlaps everything)
                zt = stage.tile([128, 8192], BF16, tag="st", bufs=2)
                nc.vector.memset(zt[:], 0.0)
                for c in range(4):
                    nc.sync.dma_start(
                        delta_a.ap()[c * 2048:(c + 1) * 2048, :], zt[:]
                    )
                    nc.sync.dma_start(
                        delta_b.ap()[c * 2048:(c + 1) * 2048, :], zt[:]
                    )

                CW1T = cappool.tile([128, ND, D], BF16)     # 16 KB/part
                for dt in range(ND):
                    st = stage.tile([128, D], F32, tag="st", bufs=2)
                    nc.sync.dma_start(st[:], cw1t.ap()[dt * 128:(dt + 1) * 128, :])
                    nc.vector.tensor_copy(CW1T[:, dt, :], st[:])
                cw2b = cappool.tile([128, ND, E], BF16)
                stc = stage.tile([128, ND, E], F32, tag="st", bufs=2)
                nc.sync.dma_start(stc[:], cw2c[:, :, :])
                nc.vector.tensor_copy(cw2b[:], stc[:])
                # b2 broadcast tile: ones(128,1) x b2 via two K=1 matmuls
                stb = stage.tile([1, D], F32, tag="st", bufs=2)
                nc.sync.dma_start(stb[:], b2d.ap().unsqueeze(0))
                for tch in range(2):
                    bp = psC.tile([128, 512], F32, tag="caph")
                    nc.tensor.matmul(
                        bp[:], ones_sb[0:1, :], stb[0:1, tch * 512:(tch + 1) * 512]
                    )
                    nc.scalar.copy(b2bc[:, tch * 512:(tch + 1) * 512], bp[:])

                # xs -> transpose -> xsT (fp32) -> scores; also cast to bf16
                xsT = p0.tile([128, ND, SL], F32, tag="xsT", bufs=1)   # 32 KB
                for tt in range(NT):
                    xtm = p0.tile([128, D], F32, tag="xs_tm")
                    nc.sync.dma_start(xtm[:], xs.ap()[tt * 128:(tt + 1) * 128, :])
                    for dt in range(ND):
                        pt = psT.tile([128, 128], F32, tag="ptr")
                        nc.tensor.transpose(
                            pt[:], xtm[:, dt * 128:(dt + 1) * 128], ident_sb[:]
                        )
                        nc.scalar.copy(xsT[:, dt, tt * 128:(tt + 1) * 128], pt[:])

                xsb = p0.tile([128, ND, SL], BF16, tag="xsb", bufs=1)  # 16 KB
                for dt in range(ND):
                    nc.vector.tensor_copy(xsb[:, dt, :], xsT[:, dt, :])

                # scores^T (8, 1024) fp32: accumulate over d tiles
                for tch in range(2):
                    scp = psA.tile([E, 512], F32, tag="sc8", bufs=2)
                    for dt in range(ND):
                        nc.tensor.matmul(
                            scp[:],
                            gw_sb[:, dt, :],
                            xsT[:, dt, tch * 512:(tch + 1) * 512],
                            start=(dt == 0),
                            stop=(dt == ND - 1),
                        )
                    nc.scalar.activation(
                        scT_sl[:, tch * 512:(tch + 1) * 512], scp[:], AF.Tanh
                    )
                nc.vector.tensor_scalar(
                    scT_sl[:], scT_sl[:], 0.5, 0.5, ALU.mult, ALU.add
                )

                # exchange scores: core e receives expert e's full 8192 scores
                nc.sync.dma_start(a2a_in.ap(), scT_sl[:])
                nc.gpsimd.collective_compute(
                    "AllToAll", ALU.bypass, replica_groups=groups,
                    ins=[a2a_in.ap()], outs=[a2a_out.ap()],
                )
                nc.sync.dma_start(
                    sc_own_b[:], a2a_out.ap().flatten().rearrange("(p f) -> p f", p=128)
                )
                nc.sync.dma_start(
                    sc16[:], a2a_out.ap().flatten().rearrange("(p f) -> p f", p=16)
                )

                # capacity predictor on own token slice (overlaps bisection)
                hcap = cappool.tile([128, ND, SL], BF16)    # 16 KB
                for it in range(ND):
                    for tch in range(2):
                        cp = psC.tile([128, 512], F32, tag="caph")
                        for dt in range(ND):
                            nc.tensor.matmul(
                                cp[:],
                                CW1T[:, dt, it * 128:(it + 1) * 128],
                                xsb[:, dt, tch * 512:(tch + 1) * 512],
                                start=(dt == 0),
                                stop=(dt == ND - 1),
                            )
                        nc.scalar.activation(
                            hcap[:, it, tch * 512:(tch + 1) * 512], cp[:],
                            AF.Gelu_apprx_tanh, bias=cb1col[:, it:it + 1],
                        )
                for tch in range(2):
                    lg = psA.tile([E, 512], F32, tag="sc8", bufs=2)
                    for it in range(ND):
                        nc.tensor.matmul(
                            lg[:],
                            cw2b[:, it, :],
                            hcap[:, it, tch * 512:(tch + 1) * 512],
                            start=(it == 0),
                            stop=(it == ND - 1),
                        )
                    nc.scalar.activation(
                        logitsT[:, tch * 512:(tch + 1) * 512], lg[:],
                        AF.Identity, bias=cb2sb[:],
                    )

                # ---- bisection: kth-largest threshold by width halving.
                # invariant count(>= lo) >= K; after N iters lo is within
                # 2^-N of the kth largest score, far inside the boundary gap.
                lo = bis.tile([128, 1], F32, tag="lo")
                nc.vector.memset(lo[:], 0.0)
                w = 1.0
                for _i in range(BISECT_ITERS):
                    w3 = w / 3.0
                    m1 = bis.tile([128, 1], F32, tag="m1")
                    m2 = bis.tile([128, 1], F32, tag="m2")
                    nc.vector.tensor_scalar_add(m1[:], lo[:], w3)
                    nc.vector.tensor_scalar_add(m2[:], lo[:], 2.0 * w3)
                    cmp = bis.tile([128, 64], F32, tag="cmp")
                    cntp = bis.tile([128, 2], F32, tag="cntp")
                    nc.vector.tensor_scalar(
                        cmp[:], sc_own_b[:], m1[:], None, ALU.is_ge,
                        ALU.add, accum_out=cntp[:, 0:1],
                    )
                    nc.vector.tensor_scalar(
                        cmp[:], sc_own_b[:], m2[:], None, ALU.is_ge,
                        ALU.add, accum_out=cntp[:, 1:2],
                    )
                    cps = psA.tile([128, 2], F32, tag="cnt", bufs=1)
                    nc.tensor.matmul(cps[:], ones_sb[:], cntp[:])
                    gef = bis.tile([128, 2], F32, tag="gef")
                    nc.vector.tensor_scalar(gef[:], cps[:], float(K) - 0.5, None, ALU.is_ge)
                    ges = bis.tile([128, 1], F32, tag="ges")
                    nc.vector.tensor_reduce(ges[:], gef[:], mybir.AxisListType.X, ALU.add)
                    nc.vector.scalar_tensor_tensor(
                        out=lo[:], in0=ges[:], scalar=w3, in1=lo[:],
                        op0=ALU.mult, op1=ALU.add,
                    )
                    w = w3

                # share thresholds (for the BCE mask)
                nc.sync.dma_start(tau_in.ap(), lo[0:1, 0:1])
                nc.gpsimd.collective_compute(
                    "AllGather", ALU.bypass, replica_groups=groups,
                    ins=[tau_in.ap()], outs=[tau_out.ap()],
                )
                nc.sync.dma_start(tau_sb[:], tau_out.ap())

                # ---- compaction: selected token ids + their scores
                mask16 = bis.tile([16, 512], I8, tag="mask16")
                nc.vector.tensor_scalar(mask16[:], sc16[:], lo[0:16, :], None, ALU.is_ge)
                midx = bis.tile([16, 512], I16, tag="midx")
                nc.vector.memset(midx[:], -1)
                nc.vector.copy_predicated(midx[:], mask16[:], iota_sb[:])
                smask = bis.tile([16, 512], F32, tag="smask")
                nc.vector.memset(smask[:], -1.0)
                nc.vector.copy_predicated(smask[:], mask16[:], sc16[:])

                lib_sg = nc.gpsimd.load_library(library_config.sparse_gather)
                idx16f = bis.tile([16, 64], F32, tag="idx16f")
                s16 = bis.tile([16, 64], F32, tag="s16")
                nf1 = bis.tile([1, 1], U32, tag="nf1")
                nf2 = bis.tile([1, 1], U32, tag="nf2")
                sg1 = nc.gpsimd.sparse_gather(idx16f[:], midx[:], num_found=nf1[:])
                sg2 = nc.gpsimd.sparse_gather(s16[:], smask[:], num_found=nf2[:])
                _dep(sg1, lib_sg)
                _dep(sg2, sg1)

                idx16 = bis.tile([16, 64], I16, tag="idx16")
                nc.vector.tensor_copy(idx16[:], idx16f[:])
                # replicate idx16 across the 8 q7 cores (sbuf->sbuf)
                for a in range(8):
                    nc.sync.dma_start(idx128[16 * a:16 * (a + 1), :], idx16[:])
                nc.sync.dma_start(
                    sel_d.ap().rearrange("(p f) -> p f", p=16), s16[:]
                )
                # s128[p, q] = score of selection slot s = q*128+p, which
                # lives at sel_d[(s%16)*64 + s//16]
                nc.sync.dma_start(
                    s128[:],
                    sel_d.ap().rearrange("(b q a) -> a b q", b=16, q=8, a=8),
                )

                # ---- BCE loss pieces (token slice x all experts)
                maskT = bis.tile([E, SL], F32, tag="maskT")
                nc.vector.tensor_scalar(maskT[:], scT_sl[:], tau_sb[:], None, ALU.is_ge)
                nc.vector.tensor_tensor(maskT[:], logitsT[:], maskT[:], ALU.mult)
                mxt = bis.tile([E, SL], F32, tag="mxt")
                nc.vector.tensor_scalar(mxt[:], logitsT[:], 0.0, None, ALU.max)
                tmp = bis.tile([E, SL], F32, tag="tmp")
                nc.scalar.activation(tmp[:], logitsT[:], AF.Abs)
                nc.scalar.activation(tmp[:], tmp[:], AF.Exp, scale=-1.0)
                nc.scalar.activation(tmp[:], tmp[:], AF.Ln, bias=1.0)
                nc.vector.tensor_tensor(mxt[:], mxt[:], tmp[:], ALU.add)
                nc.vector.tensor_tensor(mxt[:], mxt[:], maskT[:], ALU.subtract)
                part = bis.tile([E, 1], F32, tag="part")
                nc.vector.tensor_scalar(
                    tmp[:], mxt[:], 1.0, None, ALU.mult, ALU.add,
                    accum_out=part[:]
                )
                lsp = psA.tile([1, 1], F32, tag="cnt", bufs=1)
                nc.tensor.matmul(lsp[:], ones_sb[0:E, 0:1], part[:])
                lsv = bis.tile([1, 1], F32, tag="lsv")
                nc.scalar.mul(lsv[:], lsp[:], 1.0 / (BS * E))
                nc.sync.dma_start(loss_in.ap(), lsv[:])
                nc.gpsimd.collective_compute(
                    "AllReduce", ALU.add, replica_groups=groups,
                    ins=[loss_in.ap()], outs=[loss_out.ap()],
                )
                nc.sync.dma_start(loss.ap(), loss_out.ap())

                lib_mlp = nc.gpsimd.load_library(library_config.mlp)
                _dep(lib_mlp, sg2)

            # ---------------- phase 0b: W1 load + gather + LN + transpose ----
            w1_cm = tc.tile_pool(name="w1pool", bufs=1)
            w1pool = w1_cm.__enter__()
            W1T = w1pool.tile([128, ND, DD], BF16)          # 64 KB/part

            with (
                tc.tile_pool(name="xgpool", bufs=1) as xgpool,
                tc.tile_pool(name="stageb", bufs=1) as stageb,
                tc.tile_pool(name="lnp", bufs=2) as lnp,
                tc.tile_pool(name="psB", bufs=2, space="PSUM") as psB,
            ):
                for dt in range(ND):
                    st = stageb.tile([128, DD], F32, tag="stb")
                    nc.sync.dma_start(st[:], w1t.ap()[dt * 128:(dt + 1) * 128, :])
                    nc.vector.tensor_copy(W1T[:, dt, :], st[:])

                xg = xgpool.tile([128, 8, D], F32)          # 32 KB
                for a in range(4):
                    gat = nc.gpsimd.dma_gather(
                        xg[:, 2 * a:2 * a + 2, :], xfull.ap(),
                        idx128[:, 16 * a:16 * (a + 1)],
                        num_idxs=256, num_idxs_reg=256, elem_size=D,
                        queue_num=a,
                    )
                    _dep(gat, lib_mlp)

                # ---- layernorm (token-major, exact fp32 stats)
                sx = bis.tile([128, 8], F32, tag="sx")
                sxx = bis.tile([128, 8], F32, tag="sxx")
                for q in range(8):
                    scr = lnp.tile([128, D], F32, tag="lnscr")
                    nc.vector.tensor_scalar(
                        scr[:], xg[:, q, :], 1.0, None, ALU.mult,
                        ALU.add, accum_out=sx[:, q:q + 1],
                    )
                    scr2 = lnp.tile([128, D], F32, tag="lnscr2")
                    nc.scalar.activation(
                        scr2[:], xg[:, q, :], AF.Square,
                        accum_out=sxx[:, q:q + 1],
                    )
                mu = bis.tile([128, 8], F32, tag="mu")
                var = bis.tile([128, 8], F32, tag="var")
                rinv = bis.tile([128, 8], F32, tag="rinv")
                nmur = bis.tile([128, 8], F32, tag="nmur")
                nc.vector.tensor_scalar_mul(mu[:], sx[:], 1.0 / D)
                nc.vector.tensor_scalar_mul(var[:], sxx[:], 1.0 / D)
                nc.vector.tensor_tensor(nmur[:], mu[:], mu[:], ALU.mult)
                nc.vector.tensor_tensor(var[:], var[:], nmur[:], ALU.subtract)
                nc.vector.tensor_scalar_add(var[:], var[:], LN_EPS)
                nc.scalar.sqrt(var[:], var[:])
                nc.vector.reciprocal(rinv[:], var[:])
                nc.vector.tensor_tensor(nmur[:], mu[:], rinv[:], ALU.mult)
                nc.vector.tensor_scalar_mul(nmur[:], nmur[:], -1.0)
                for q in range(8):
                    nc.scalar.activation(
                        xg[:, q, :], xg[:, q, :], AF.Identity,
                        bias=nmur[:, q:q + 1], scale=rinv[:, q:q + 1],
                    )
                    for dt in range(ND):
                        pt2 = psB.tile([128, 128], F32, tag="ptr2")
                        nc.tensor.transpose(
                            pt2[:], xg[:, q, dt * 128:(dt + 1) * 128], ident_sb[:]
                        )
                        nc.scalar.activation(
                            xlnT[:, dt, q * 128:(q + 1) * 128], pt2[:],
                            AF.Identity, bias=bcol[:, dt:dt + 1],
                            scale=gcol[:, dt:dt + 1],
                        )

            bis_cm.__exit__(None, None, None)

            # ---------------- phase 1: fc1 for all tokens ----------------
            hs_cm = tc.tile_pool(name="hspool", bufs=1, side="right")
            hspool = hs_cm.__enter__()
            hs = hspool.tile([128, NF, K], BF16)            # 64 KB/part

            with (
                tc.tile_pool(name="psH", bufs=3, space="PSUM") as psH,
                tc.tile_pool(name="hsg", bufs=3) as hsgp,
            ):
                for c in range(2):
                    for ft in range(NF):
                        hp = psH.tile([128, 512], F32, tag="hp")
                        for dt in range(ND):
                            nc.tensor.matmul(
                                hp[:],
                                W1T[:, dt, ft * 128:(ft + 1) * 128],
                                xlnT[:, dt, c * 512:(c + 1) * 512],
                                start=(dt == 0),
                                stop=(dt == ND - 1),
                            )
                        nc.scalar.activation(
                            hs[:, ft, c * 512:(c + 1) * 512], hp[:],
                            AF.Gelu_apprx_tanh, bias=b1col[:, ft:ft + 1],
                        )

            w1_cm.__exit__(None, None, None)
            xln_cm.__exit__(None, None, None)

            # ------------- phase 2: fc2 + scatter-add -------------
            with (
                tc.tile_pool(name="w2pool", bufs=1) as w2pool,
                tc.tile_pool(name="stage2", bufs=2) as stage2,
                tc.tile_pool(name="ypool", bufs=1) as ypool,
                tc.tile_pool(name="psY", bufs=1, space="PSUM") as psY,
            ):
                W2T = w2pool.tile([128, NF, D], BF16)       # 64 KB/part
                for ft in range(NF):
                    st2 = stage2.tile([128, D], F32, tag="w2stage")
                    nc.sync.dma_start(st2[:], w2t.ap()[ft * 128:(ft + 1) * 128, :])
                    nc.vector.tensor_copy(W2T[:, ft, :], st2[:])

                for dc, (ddst, drs) in enumerate(
                    [(delta_a, delta_rs_a), (delta_b, delta_rs_b)]
                ):
                    yps = [
                        psY.tile([128, 512], F32, tag=f"yp{g}", bufs=1,
                                 name=f"ypt{dc}_{g}")
                        for g in range(8)
                    ]
                    for c in range(2):
                        for ft in range(NF):
                            for tt in range(4):
                                nc.tensor.matmul(
                                    yps[c * 4 + tt][:],
                                    hs[:, ft, c * 512 + tt * 128:c * 512 + (tt + 1) * 128],
                                    W2T[:, ft, dc * 512:(dc + 1) * 512],
                                    start=(ft == 0),
                                    stop=(ft == NF - 1),
                                )
                    ysb = ypool.tile([128, 8, D // 2], BF16, tag="ysb",
                                     name=f"ysb{dc}")
                    ytmp = ypool.tile([128, 512], F32, tag="ytmp", bufs=2,
                                      name=f"ytmp{dc}")
                    for g in range(8):
                        nc.vector.tensor_tensor(
                            ytmp[:], yps[g][:],
                            b2bc[:, dc * 512:(dc + 1) * 512], ALU.add,
                        )
                        nc.vector.tensor_scalar(
                            ysb[:, g, :], ytmp[:], s128[:, g:g + 1],
                            None, ALU.mult,
                        )
                    for a in range(4):
                        scat = nc.gpsimd.dma_scatter_add(
                            ddst.ap(), ysb[:, 2 * a:2 * a + 2, :],
                            idx128[:, 16 * a:16 * (a + 1)],
                            num_idxs=256, num_idxs_reg=256, elem_size=D // 2,
                            queue_num=a,
                        )
                        _dep(scat, lib_mlp)
                    nc.gpsimd.collective_compute(
                        "ReduceScatter", ALU.add, replica_groups=groups,
                        ins=[ddst.ap()], outs=[drs.ap()],
                    )

            hs_cm.__exit__(None, None, None)

            # ---------------- phase 3: combine ----------------
            with tc.tile_pool(name="fin", bufs=3) as fin:
                for tt in range(NT):
                    dl = fin.tile([128, D], BF16, tag="dl")
                    nc.sync.dma_start(
                        dl[:, 0:512], delta_rs_a.ap()[tt * 128:(tt + 1) * 128, :]
                    )
                    nc.sync.dma_start(
                        dl[:, 512:1024], delta_rs_b.ap()[tt * 128:(tt + 1) * 128, :]
                    )
                    xr = fin.tile([128, D], F32, tag="xr")
                    nc.sync.dma_start(xr[:], xs.ap()[tt * 128:(tt + 1) * 128, :])
                    ov = fin.tile([128, D], F32, tag="ov")
                    nc.vector.tensor_tensor(ov[:], dl[:], xr[:], ALU.add)
                    nc.sync.dma_start(out_slice.ap()[tt * 128:(tt + 1) * 128, :], ov[:])

    nc.compile()
    return nc


def make_in_maps(inputs):
    """Shard the full inputs into 8 per-core input maps (host-side data
    movement only: slicing, transposes, trivial constants)."""
    x = np.ascontiguousarray(np.asarray(inputs["x"], dtype=np.float32)).reshape(BS, D)
    gate_w = np.asarray(inputs["gate_w"], dtype=np.float32)
    cap_w1 = np.asarray(inputs["cap_w1"], dtype=np.float32)
    cap_b1 = np.asarray(inputs["cap_b1"], dtype=np.float32)
    cap_w2 = np.asarray(inputs["cap_w2"], dtype=np.float32)
    cap_b2 = np.asarray(inputs["cap_b2"], dtype=np.float32)
    norm_w = np.asarray(inputs["norm_w"], dtype=np.float32)
    norm_b = np.asarray(inputs["norm_b"], dtype=np.float32)
    fc1s = np.asarray(inputs["fc1s"], dtype=np.float32)
    b1s = np.asarray(inputs["b1s"], dtype=np.float32)
    fc2s = np.asarray(inputs["fc2s"], dtype=np.float32)
    b2s = np.asarray(inputs["b2s"], dtype=np.float32)

    cw1t = np.ascontiguousarray(cap_w1.T)
    # [p, dt, e] layouts for per-partition stationary tiles
    gwc = np.ascontiguousarray(gate_w.T.reshape(8, 128, E).transpose(1, 0, 2))
    cw2c = np.ascontiguousarray(cap_w2.T.reshape(8, 128, E).transpose(1, 0, 2))
    nwc = np.ascontiguousarray(norm_w.reshape(8, 128).T)
    nbc = np.ascontiguousarray(norm_b.reshape(8, 128).T)
    cb1c = np.ascontiguousarray(cap_b1.reshape(8, 128).T)
    ident = np.eye(128, dtype=np.float32)
    ones = np.ones((128, 128), dtype=np.float32)
    # iota16[p, f] = p*512 + f  (token id at position (p, f) of the
    # linearly-loaded sc16 tile)
    iota16 = (np.arange(16, dtype=np.int16)[:, None] * 512
              + np.arange(512, dtype=np.int16)[None, :])
    iota16 = np.ascontiguousarray(iota16)

    in_maps = []
    for r in range(8):
        in_maps.append({
            "xs": np.ascontiguousarray(x[r * SL:(r + 1) * SL, :]),
            "xfull": x,
            "gwc": gwc,
            "w1t": np.ascontiguousarray(fc1s[r].T),
            "w2t": np.ascontiguousarray(fc2s[r].T),
            "b1c": np.ascontiguousarray(b1s[r].reshape(32, 128).T),
            "b2d": np.ascontiguousarray(b2s[r]),
            "cw1t": cw1t,
            "cb1c": cb1c,
            "cw2c": cw2c,
            "cb2d": cap_b2,
            "nwc": nwc,
            "nbc": nbc,
            "identd": ident,
            "onesd": ones,
            "iotad": iota16,
        })
    return in_maps


def assemble(results):
    out = np.concatenate(
        [results[r]["out_slice"] for r in range(8)], axis=0
    ).reshape(4, 2048, D)
    cap_loss = np.float32(results[0]["loss"][0, 0])
    return out, cap_loss


def kernel(**inputs):
    if "nc" not in _CACHE:
        _CACHE["nc"] = build_program()
    nc = _CACHE["nc"]
    in_maps = make_in_maps(inputs)
    res = run_bass_kernel_spmd(nc, in_maps, core_ids=list(range(8)))
    return assemble(res.results)
